# revision 2
# baseline (speedup 1.0000x reference)
"""Multi-head attention (B=2, S=1024, D=768, H=12) on 8 TRN2 NeuronCores.

Sharding: batch x head-group. Core c handles batch b = c // 4 and heads
3*(c%4) .. 3*(c%4)+2. Each core computes q/k/v projections for its heads,
attention with additive mask + key-padding mask, and the partial output
projection through its rows of Wo. Host sums the 4 partials per batch and
adds bo.

Device-side layout: everything runs in "transposed world". The attention
mask is transposed on the host per (b, h) so logits are computed as
S^T [SK_part, SQ_free] = k^T.T @ q^T directly in the layout the AV matmul
needs (contraction over SK on partitions). The key-padding mask becomes a
per-partition bias of the exp() on the scalar engine. Softmax denominators
come for free from an extra ones-column in v (row 64 of ctx_aug^T).
No PE transposes of the attention weights are needed.

Matmuls run in float32r (TF32-like, ~1.5e-4 relative rounding on inputs,
fp32 PSUM accumulation) for 4x PE throughput vs fp32.
"""

import numpy as np

B, SQ, SK, D, H = 2, 1024, 1024, 768, 12
DH = D // H            # 64
HPC = 3                # heads per core
N_CORES = 8
GPB = 4                # head-groups (cores) per batch
KT = 7                 # k-tiles over the augmented contraction dim 769
NEG = -1.0e30

_CACHE = {}


def _build():
    import concourse.tile as tile
    import concourse.mybir as mybir
    from concourse import bacc

    f32 = mybir.dt.float32
    f32r = mybir.dt.float32r
    AF = mybir.ActivationFunctionType

    nc = bacc.Bacc("TRN2", target_bir_lowering=False, debug=False,
                   num_devices=N_CORES)

    qT = nc.dram_tensor("qT", [769, SQ], f32r, kind="ExternalInput").ap()
    kT = nc.dram_tensor("kT", [769, SK], f32r, kind="ExternalInput").ap()
    vT = nc.dram_tensor("vT", [769, SK], f32r, kind="ExternalInput").ap()
    # WqA: cols 0:192 = Wq (scaled, + bias row), cols 192:384 = Wk (+ bias row)
    WqA = nc.dram_tensor("WqA", [769, 384], f32r, kind="ExternalInput").ap()
    WvA = nc.dram_tensor("WvA", [769, 256], f32r, kind="ExternalInput").ap()
    WoR = nc.dram_tensor("WoR", [HPC, DH, D], f32r, kind="ExternalInput").ap()
    maskT = nc.dram_tensor("maskT", [HPC, SK, SQ], f32, kind="ExternalInput").ap()
    padc = nc.dram_tensor("padc", [128, 8], f32, kind="ExternalInput").ap()
    ones64 = nc.dram_tensor("ones64", [1, DH], f32r, kind="ExternalInput").ap()
    out_d = nc.dram_tensor("out", [SQ, D], f32, kind="ExternalOutput").ap()

    with tile.TileContext(nc) as tc:
        with (
            tc.tile_pool(name="consts", bufs=1) as cp,
            tc.tile_pool(name="xt", bufs=9) as xtp,
            tc.tile_pool(name="qk", bufs=1) as qkp,
            tc.tile_pool(name="vv", bufs=1) as vvp,
            tc.tile_pool(name="mask", bufs=4) as mkp,
            tc.tile_pool(name="lt", bufs=3) as ltp,
            tc.tile_pool(name="pt", bufs=4) as ptp,
            tc.tile_pool(name="norm", bufs=2) as nmp,
            tc.tile_pool(name="outs", bufs=3) as otp,
            tc.tile_pool(name="ps", bufs=4, space="PSUM") as ps,
        ):
            # ---- constants ----
            wq, wv = [], []
            for t in range(KT):
                p = 128 if t < 6 else 1
                w1 = cp.tile([p, 384], f32r, tag=f"wq{t}")
                nc.sync.dma_start(w1[:], WqA[t * 128:t * 128 + p, :])
                wq.append(w1)
                w3 = cp.tile([p, 256], f32r, tag=f"wv{t}")
                nc.sync.dma_start(w3[:], WvA[t * 128:t * 128 + p, :])
                wv.append(w3)
            wo = []
            for j in range(HPC):
                w = cp.tile([DH, D], f32r, tag=f"wo{j}")
                nc.sync.dma_start(w[:], WoR[j])
                wo.append(w)
            pad = cp.tile([128, 8], f32, tag="pad")
            nc.sync.dma_start(pad[:], padc)
            o64 = cp.tile([1, DH], f32r, tag="o64")
            nc.sync.dma_start(o64[:], ones64)

            def load_x(x_dram):
                ts_ = []
                for t in range(KT):
                    p = 128 if t < 6 else 1
                    xt_t = xtp.tile([p, SQ], f32r, tag="xt")
                    nc.sync.dma_start(xt_t[:], x_dram[t * 128:t * 128 + p, :])
                    ts_.append(xt_t)
                return ts_

            # ---- q^T / k^T projections: [64*HPC, S] as t0 (heads 0,1) + t1 ----
            def proj_qk(xts, col0, name):
                outs = []
                for c, rows in ((0, 128), (1, DH)):
                    dst = qkp.tile([rows, SQ], f32r, tag=f"{name}{c}")
                    pps = ps.tile([128, SQ], f32, tag="ps")
                    for t in range(KT):
                        lhs = wq[t][:, col0 + c * 128: col0 + c * 128 + rows]
                        for n in range(2):
                            nc.tensor.matmul(
                                pps[0:rows, n * 512:(n + 1) * 512],
                                lhs, xts[t][:, n * 512:(n + 1) * 512],
                                start=(t == 0), stop=(t == KT - 1))
                    nc.scalar.copy(dst[:], pps[0:rows, :])
                    outs.append(dst)
                return outs

            qx = load_x(qT)
            qrows = proj_qk(qx, 0, "q")
            kx = load_x(kT)
            krows = proj_qk(kx, 192, "k")

            # ---- v projection: [SK, 65*HPC] (col 64 of each head block = ones)
            vx = load_x(vT)
            vtiles = []
            for i in range(8):
                vps = ps.tile([128, 256], f32, tag="ps")
                for t in range(KT):
                    nc.tensor.matmul(
                        vps[:], vx[t][:, i * 128:(i + 1) * 128], wv[t][:],
                        start=(t == 0), stop=(t == KT - 1))
                vt = vvp.tile([128, 65 * HPC], f32r, tag=f"v{i}")
                nc.scalar.copy(vt[:], vps[:, 0:65 * HPC])
                vtiles.append(vt)

            # ---- attention per head ----
            ctxn = []
            for j in range(HPC):
                if j < 2:
                    qsrc = qrows[0][j * DH:(j + 1) * DH, :]
                    ksrc = krows[0][j * DH:(j + 1) * DH, :]
                else:
                    qsrc = qrows[1][:, :]
                    ksrc = krows[1][:, :]
                ctx = ps.tile([65, SQ], f32, tag="ps")
                for i in range(8):
                    sps = ps.tile([128, SQ], f32, tag="ps")
                    for n in range(2):
                        nc.tensor.matmul(
                            sps[:, n * 512:(n + 1) * 512],
                            ksrc[:, i * 128:(i + 1) * 128],
                            qsrc[:, n * 512:(n + 1) * 512],
                            start=True, stop=True)
                    mk = mkp.tile([128, SQ], f32, tag="mask")
                    nc.sync.dma_start(mk[:], maskT[j, i * 128:(i + 1) * 128, :])
                    lt = ltp.tile([128, SQ], f32, tag="lt")
                    nc.vector.tensor_add(lt[:], sps[:], mk[:])
                    pt = ptp.tile([128, SQ], f32r, tag="pt")
                    nc.scalar.activation(pt[:], lt[:], AF.Exp,
                                         bias=pad[:, i:i + 1], scale=1.0)
                    for n in range(2):
                        nc.tensor.matmul(
                            ctx[:, n * 512:(n + 1) * 512],
                            vtiles[i][:, j * 65:(j + 1) * 65],
                            pt[:, n * 512:(n + 1) * 512],
                            start=(i == 0), stop=(i == 7))
                # row 64 of ctx = softmax denominators; partition-shifted copy
                srow = nmp.tile([1, SQ], f32r, tag="srow")
                nc.scalar.copy(srow[:], ctx[DH:DH + 1, :])
                rb = ps.tile([DH, SQ], f32, tag="ps")
                for n in range(2):
                    nc.tensor.matmul(rb[:, n * 512:(n + 1) * 512],
                                     o64[:], srow[:, n * 512:(n + 1) * 512],
                                     start=True, stop=True)
                rbr = nmp.tile([DH, SQ], f32, tag="rbr")
                nc.vector.reciprocal(rbr[:], rb[:])
                cn = nmp.tile([DH, SQ], f32r, tag=f"cn{j}")
                nc.vector.tensor_mul(cn[:], ctx[0:DH, :], rbr[:])
                ctxn.append(cn)

            # ---- output projection ----
            for t in range(8):
                ops = ps.tile([128, D], f32, tag="ps")
                for n0, nw in ((0, 512), (512, 256)):
                    for j in range(HPC):
                        nc.tensor.matmul(
                            ops[:, n0:n0 + nw],
                            ctxn[j][:, t * 128:(t + 1) * 128],
                            wo[j][:, n0:n0 + nw],
                            start=(j == 0), stop=(j == HPC - 1))
                ot = otp.tile([128, D], f32, tag="ot")
                nc.vector.tensor_copy(ot[:], ops[:, 0:D])
                nc.sync.dma_start(out_d[t * 128:(t + 1) * 128, :], ot[:])

    nc.compile()
    return nc


def prep_inputs(value, key, query, key_padding_mask, attn_mask,
                Wq, Wk, Wv, Wo, bq, bk, bv, bo):
    f = np.float32
    value = np.asarray(value, f)
    key = np.asarray(key, f)
    query = np.asarray(query, f)
    key_padding_mask = np.asarray(key_padding_mask)
    attn_mask = np.asarray(attn_mask, f)
    Wq, Wk, Wv, Wo = (np.asarray(w, f) for w in (Wq, Wk, Wv, Wo))
    bq, bk, bv = (np.asarray(x, f) for x in (bq, bk, bv))

    scale = f(1.0 / np.sqrt(DH))
    ones_row = np.ones((1, SQ), f)
    xT = {}
    for b in range(B):
        xT[("q", b)] = np.concatenate(
            [np.ascontiguousarray(query[b].T), ones_row]).astype(f)
        xT[("k", b)] = np.concatenate(
            [np.ascontiguousarray(key[b].T), ones_row]).astype(f)
        xT[("v", b)] = np.concatenate(
            [np.ascontiguousarray(value[b].T), ones_row]).astype(f)
    maskT_all = np.ascontiguousarray(attn_mask.transpose(0, 1, 3, 2))
    pad_all = np.where(key_padding_mask, f(0), f(NEG)).astype(f)  # [B, SK]

    in_maps = []
    for c in range(N_CORES):
        b, g = divmod(c, GPB)
        h0 = g * HPC
        cols = slice(h0 * DH, (h0 + HPC) * DH)
        WqA = np.zeros((769, 384), f)
        WqA[:768, 0:192] = Wq[:, cols] * scale
        WqA[768, 0:192] = bq[cols] * scale
        WqA[:768, 192:384] = Wk[:, cols]
        WqA[768, 192:384] = bk[cols]
        WvA = np.zeros((769, 256), f)
        for j in range(HPC):
            hc = slice((h0 + j) * DH, (h0 + j + 1) * DH)
            WvA[:768, j * 65:j * 65 + DH] = Wv[:, hc]
            WvA[768, j * 65:j * 65 + DH] = bv[hc]
            WvA[768, j * 65 + DH] = 1.0
        WoR = np.ascontiguousarray(Wo[cols].reshape(HPC, DH, D)).astype(f)
        in_maps.append({
            "qT": xT[("q", b)],
            "kT": xT[("k", b)],
            "vT": xT[("v", b)],
            "WqA": WqA,
            "WvA": WvA,
            "WoR": WoR,
            "maskT": np.ascontiguousarray(maskT_all[b, h0:h0 + HPC]),
            "padc": np.ascontiguousarray(pad_all[b].reshape(8, 128).T),
            "ones64": np.ones((1, DH), f),
        })
    return in_maps


def get_nc():
    if "nc" not in _CACHE:
        _CACHE["nc"] = _build()
    return _CACHE["nc"]


def assemble(results, bo):
    out = np.zeros((B, SQ, D), np.float32)
    for c in range(N_CORES):
        out[c // GPB] += results[c]["out"]
    return out + np.asarray(bo, np.float32)


def kernel(value, key, query, key_padding_mask, attn_mask,
           Wq, Wk, Wv, Wo, bq, bk, bv, bo, **extra):
    from concourse.bass_utils import run_bass_kernel_spmd

    nc = get_nc()
    in_maps = prep_inputs(value, key, query, key_padding_mask, attn_mask,
                          Wq, Wk, Wv, Wo, bq, bk, bv, bo)
    res = run_bass_kernel_spmd(nc, in_maps, core_ids=list(range(N_CORES)),
                               **_CACHE.get("run_kwargs", {}))
    _CACHE["last_results"] = res
    return assemble(res.results, bo)
